# revision 1
# baseline (speedup 1.0000x reference)
"""Dynamic filter layer on 8 trn2 NeuronCores.

out[b,i,j,c] = sum_{di,dj} x[b,i+di,j+dj,c] * flow[b,i,j,di*K+dj]

B=8, H=W=256, C=64, K=5, Ho=Wo=252. Sharding: data-parallel over batch,
one sample per core (SPMD, no collectives).

Per-core algorithm (v3): per tap, on a [rows, 64 cols, 64 ch] chunk:
    tmp_t = x_win * flow_bcast     (DVE tensor_tensor mult; the flow value
                                    is broadcast along c via a step-0 AP)
    psum += S_di.T @ tmp_t         (TensorE matmul, identity shifted by di:
                                    lhsT = I128[:, di:di+124] — does BOTH the
                                    di row-shift and the 25-tap accumulation
                                    in PSUM, for free on the idle PE)
    out_chunk = psum               (ScalarE copy PSUM->SBUF, then DMA)

The row shift lives in the PE weight slice, so only ONE x tile and 5
cheap row-shifted flow copies are needed (engine APs must start at
partition 0, so shifts cannot be partition offsets). tmp rows with no
valid flow row (top block, k < di) are zeroed so 0*x stays finite.
DVE does only 25 long-FD mults per chunk (~5.4us each); adds cost zero
DVE time. The final 4 output rows (252 = 2*124 + 4) use the same scheme
transposed (partition = output column, dj shift via 5 x copies, di on
the free axis).
"""

import numpy as np

H = 256
W = 256
C = 64
K = 5
HO = H - K + 1  # 252
WO = W - K + 1  # 252
NCORES = 8
JW = 64  # column chunk width
BANK_J = 8  # 8 cols x 64 ch = 512 f32 = one PSUM bank

_nc_cache = {}


def _build(reps=1, n_gp=0):
    """reps>1 wraps the whole body in a HW loop (timing calibration only).
    n_gp>0 moves that many taps' multiplies to GpSimd."""
    global _nc_cache
    key = (reps, n_gp)
    if key in _nc_cache:
        return _nc_cache[key]

    import contextlib

    import concourse.bacc as bacc
    import concourse.bass as bass
    import concourse.tile as tile
    from concourse import mybir
    from concourse.masks import make_identity

    f32 = mybir.dt.float32
    mult = mybir.AluOpType.mult
    add = mybir.AluOpType.add

    nc = bacc.Bacc(None, target_bir_lowering=False)
    x = nc.dram_tensor("x", [H, W, C], f32, kind="ExternalInput")
    flow = nc.dram_tensor("flow", [HO, WO, K * K], f32, kind="ExternalInput")
    out = nc.dram_tensor("out", [HO, WO, C], f32, kind="ExternalOutput")

    fbufs = 1 if n_gp else 2

    with tile.TileContext(nc) as tc:
        with (
            tc.tile_pool(name="cst", bufs=1) as cst,
            tc.tile_pool(name="xp", bufs=2) as xp,
            tc.tile_pool(name="fp", bufs=fbufs) as fp,
            tc.tile_pool(name="td", bufs=4) as td,
            tc.tile_pool(name="tg", bufs=2) as tg,
            tc.tile_pool(name="sp", bufs=2) as sp,
            tc.tile_pool(name="pp", bufs=1, space="PSUM") as pp,
        ):
            ident = cst.tile([128, 128], f32, tag="ident")
            make_identity(nc, ident)

            gp_taps = set(range(K * K - n_gp, K * K))

            with tc.For_i(0, reps, 1) if reps > 1 else contextlib.nullcontext():
                # --- main blocks: out rows [0,124) and [124,248) ---
                for i0 in (0, 124):
                    for j0 in range(0, WO, JW):
                        jw = min(JW, WO - j0)
                        xw = min(jw + K - 1, W - j0)
                        xt = xp.tile([128, JW + K - 1, C], f32, tag="x")
                        nc.sync.dma_start(
                            out=xt[:, :xw, :],
                            in_=x[i0 : i0 + 128, j0 : j0 + xw, :],
                        )
                        # fc5[di][k] = flow[i0 + k - di]; rows k < di of the
                        # top block have no source row -> zeroed.
                        fc5 = []
                        for di in range(K):
                            # only taps di*K..di*K+4 are read from this copy
                            ft = fp.tile([128, JW, K], f32, tag=f"f{di}")
                            lo = i0 - di
                            ts0 = di * K
                            if lo >= 0:
                                nc.sync.dma_start(
                                    out=ft[:, :jw, :],
                                    in_=flow[
                                        lo : lo + 128, j0 : j0 + jw,
                                        ts0 : ts0 + K,
                                    ],
                                )
                            else:
                                nc.gpsimd.memset(ft[: -lo, :jw, :], 0.0)
                                nc.sync.dma_start(
                                    out=ft[-lo:, :jw, :],
                                    in_=flow[
                                        0 : 128 + lo, j0 : j0 + jw,
                                        ts0 : ts0 + K,
                                    ],
                                )
                            fc5.append(ft)

                        ps = pp.tile([124, JW, C], f32, tag="ps")
                        for t in range(K * K):
                            di, dj = divmod(t, K)
                            tmp = td.tile([128, JW, C], f32, tag="tmpd")
                            fb = fc5[di][:, :jw, dj : dj + 1].to_broadcast(
                                [128, jw, C]
                            )
                            nc.vector.tensor_tensor(
                                out=tmp[:, :jw, :],
                                in0=xt[:, dj : dj + jw, :],
                                in1=fb,
                                op=mult,
                            )
                            for jj in range(0, jw, BANK_J):
                                njw = min(BANK_J, jw - jj)
                                nc.tensor.matmul(
                                    ps[:, jj : jj + njw, :],
                                    ident[:, di : di + 124],
                                    tmp[:, jj : jj + njw, :],
                                    start=(t == 0),
                                    stop=(t == K * K - 1),
                                )
                        stage = sp.tile([124, JW, C], f32, tag="stage")
                        nc.scalar.copy(out=stage[:, :jw, :], in_=ps[:, :jw, :])
                        nc.sync.dma_start(
                            out=out[i0 : i0 + 124, j0 : j0 + jw, :],
                            in_=stage[:, :jw, :],
                        )


                # --- strip: out rows [248,252), transposed (partition=j) ---
                for j0, P in ((0, 124), (124, 124), (248, 4)):
                    xs5 = []
                    for dj in range(K):
                        xs = fp.tile([P, 8, C], f32, tag=f"f{dj}")
                        nc.sync.dma_start(
                            out=xs,
                            in_=x[
                                HO - 4 : HO + 4, j0 + dj : j0 + dj + P, :
                            ].rearrange("r j c -> j r c"),
                        )
                        xs5.append(xs)
                    fs = xp.tile([P, 4, K * K], f32, tag="x")
                    nc.sync.dma_start(
                        out=fs,
                        in_=flow[HO - 4 : HO, j0 : j0 + P, :].rearrange(
                            "i j t -> j i t"
                        ),
                    )
                    ps_s = pp.tile([P, 4, C], f32, tag="ps")
                    for t in range(K * K):
                        di, dj = divmod(t, K)
                        tmp = td.tile([P, 4, C], f32, tag="tmpd")
                        fb = fs[:, :, t : t + 1].to_broadcast([P, 4, C])
                        nc.vector.tensor_tensor(
                            out=tmp,
                            in0=xs5[dj][:, di : di + 4, :],
                            in1=fb,
                            op=mult,
                        )
                        nc.tensor.matmul(
                            ps_s[:, :, :],
                            ident[:P, :P],
                            tmp[:, :, :],
                            start=(t == 0),
                            stop=(t == K * K - 1),
                        )
                    stage = sp.tile([P, 4, C], f32, tag="stage")
                    nc.scalar.copy(out=stage, in_=ps_s)
                    nc.sync.dma_start(
                        out=out[HO - 4 : HO, j0 : j0 + P, :].rearrange(
                            "i j c -> j i c"
                        ),
                        in_=stage,
                    )

    nc.finalize()
    _nc_cache[key] = nc
    return nc


def _run(x, flow, trace=False):
    """x: [8,H,W,C] f32, flow: [8,HO,WO,25] f32 -> (out [8,HO,WO,C], results)"""
    from concourse.bass_utils import run_bass_kernel_spmd

    nc = _build()
    in_maps = [
        {
            "x": np.ascontiguousarray(x[b], dtype=np.float32),
            "flow": np.ascontiguousarray(flow[b], dtype=np.float32),
        }
        for b in range(NCORES)
    ]
    res = run_bass_kernel_spmd(
        nc, in_maps, core_ids=list(range(NCORES)), trace=trace
    )
    out = np.stack([r["out"] for r in res.results], axis=0)
    return out, res


def kernel(x, flow, ksize=None, **_unused):
    x = np.asarray(x, dtype=np.float32)
    flow = np.asarray(flow, dtype=np.float32)
    out, _ = _run(x, flow, trace=False)
    return out



# revision 2
# speedup vs baseline: 1.3531x; 1.3531x over previous
"""Dynamic filter layer on 8 trn2 NeuronCores — v4 (bf16 + 4x DVE + Pool split).

out[b,i,j,c] = sum_{di,dj} x[b,i+di,j+dj,c] * flow[b,i,j,di*K+dj]

B=8, H=W=256, C=64, K=5, Ho=Wo=252. Sharding: data-parallel over batch,
one sample per core (SPMD, no collectives).

v4 design (per core), all elementwise math in bf16:
  - Host preps x as bf16 and flow as "dup" bf16 fd[i,j,2t]=fd[i,j,2t+1]=
    flow[i,j,t]. The dup pair lets the DVE read the per-(i,j) filter value
    with an innermost [2]-stride-1 access pattern ([jw, 32x(stride0), 2]),
    which keeps the 2x_1p DVE perf mode engaged (a plain stride-0 channel
    broadcast would drop it to 1x). NB scalar_tensor_tensor reports NO
    perf modes, so plain tensor_tensor (2x-capable) is the fastest
    2-tensor DVE op.
  - Per tap, tmp = x_window * flow_bcast. Taps sharing the same di may be
    pre-combined (adds) into group tensors; the PE row-shift-accumulates
    the groups into PSUM via bf16 shifted-identity matmuls (1 cycle/row
    vs 4 for the old fp32 ones). 23 groups / 2 pre-adds balances DVE
    (16 ops @ 2x) / Pool (11 ops @ 1x) / PE (23 streams).
  - Column chunks of 32 (psum tile = 4 banks) so two chunks alternate
    PSUM halves: ACT's psum->sbuf copy overlaps the next chunk's matmuls.
  - Output staged as bf16 (host upcasts to f32): halves the out DMA.
Final 4 output rows (252 = 2*124 + 4) use the transposed scheme
(partition = output column, dj via 5 x copies, di on the free axis).
"""

import numpy as np

H = 256
W = 256
C = 64
K = 5
HO = H - K + 1  # 252
WO = W - K + 1  # 252
NCORES = 8
JW = 32  # column chunk width; psum tile [124, JW, C] f32 = 4 PSUM banks
BANK_J = 8  # 8 cols x 64 ch = 512 f32 = one PSUM bank

# Per-di tap grouping (taps are dj indices). Taps inside one group are
# pre-added on DVE/Pool; each group is one PE shift-accumulate stream.
GROUPS = {
    0: [[0], [1], [2], [3], [4]],
    1: [[0], [1], [2], [3], [4]],
    2: [[0], [1], [2], [3], [4]],
    3: [[0, 1], [2], [3], [4]],
    4: [[0, 1], [2], [3], [4]],
}
# (di, group_idx) handled on Pool (11 single-tap groups); rest on DVE.
POOL_GROUPS = {
    (0, 1), (0, 2), (0, 3),
    (1, 1), (1, 2), (1, 3),
    (2, 1), (2, 2), (2, 3),
    (3, 3), (4, 3),
}
N_STREAMS = sum(len(g) for g in GROUPS.values())  # 18

_nc_cache = {}


def _build(reps=1):
    """reps>1 wraps the whole body in a HW loop (timing calibration only)."""
    global _nc_cache
    if reps in _nc_cache:
        return _nc_cache[reps]

    import contextlib

    import concourse.bacc as bacc
    import concourse.tile as tile
    from concourse import mybir
    from concourse.masks import make_identity

    f32 = mybir.dt.float32
    bf16 = mybir.dt.bfloat16
    mult = mybir.AluOpType.mult
    add = mybir.AluOpType.add

    nc = bacc.Bacc(None, target_bir_lowering=False)
    x = nc.dram_tensor("x", [H, W, C], bf16, kind="ExternalInput")
    fd = nc.dram_tensor("fd", [HO, WO, 2 * K * K], bf16, kind="ExternalInput")
    out = nc.dram_tensor("out", [HO, WO, C], bf16, kind="ExternalOutput")

    def fb(ft, dj, jw, rows=128, mid=4):
        # [rows, jw, 2] dup pair -> [rows, jw, C//2, 2]: innermost stride-1
        # pair keeps DVE 4x; middle dim broadcasts across channels.
        return (
            ft[:rows, :jw, 2 * dj : 2 * dj + 2]
            .unsqueeze(2)
            .to_broadcast([rows, jw, C // 2, 2])
        )

    def stt(eng, out_, in0, in1, op1):
        eng.tensor_tensor(out=out_, in0=in0, in1=in1, op=op1)

    with tile.TileContext(nc) as tc:
        with (
            tc.tile_pool(name="cst", bufs=1) as cst,
            tc.tile_pool(name="xp", bufs=2) as xp,
            tc.tile_pool(name="fp", bufs=2) as fp,
            tc.tile_pool(name="td", bufs=1) as td,
            tc.tile_pool(name="sp", bufs=2) as sp,
            tc.tile_pool(name="pp", bufs=2, space="PSUM") as pp,
        ):
            ident = cst.tile([128, 128], bf16, tag="ident")
            make_identity(nc, ident)

            with tc.For_i(0, reps, 1) if reps > 1 else contextlib.nullcontext():
                # --- main blocks: out rows [0,124) and [124,248) ---
                for i0 in (0, 124):
                    for j0 in range(0, WO, JW):
                        jw = min(JW, WO - j0)
                        xw = min(jw + K - 1, W - j0)
                        xt = xp.tile([128, JW + K - 1, C], bf16, tag="x")
                        nc.sync.dma_start(
                            out=xt[:, :xw, :],
                            in_=x[i0 : i0 + 128, j0 : j0 + xw, :],
                        )
                        # fc5[di][k] = fd[i0 + k - di] (taps di*K..di*K+4,
                        # dup'd); rows k < di of the top block are zeroed.
                        fc5 = []
                        for di in range(K):
                            ft = fp.tile([128, JW, 2 * K], bf16, tag=f"f{di}")
                            lo = i0 - di
                            ts0 = 2 * K * di
                            if lo >= 0:
                                nc.sync.dma_start(
                                    out=ft[:, :jw, :],
                                    in_=fd[
                                        lo : lo + 128, j0 : j0 + jw,
                                        ts0 : ts0 + 2 * K,
                                    ],
                                )
                            else:
                                nc.gpsimd.memset(ft[: -lo, :jw, :], 0.0)
                                nc.sync.dma_start(
                                    out=ft[-lo:, :jw, :],
                                    in_=fd[
                                        0 : 128 + lo, j0 : j0 + jw,
                                        ts0 : ts0 + 2 * K,
                                    ],
                                )
                            fc5.append(ft)

                        ps = pp.tile([124, JW, C], f32, tag="ps")
                        stream = 0
                        for di in range(K):
                            for gi, taps in enumerate(GROUPS[di]):
                                eng = (
                                    nc.gpsimd
                                    if (di, gi) in POOL_GROUPS
                                    else nc.vector
                                )
                                g = td.tile(
                                    [128, JW, C], bf16, tag="g", bufs=6
                                )
                                stt(
                                    eng,
                                    g[:, :jw, :],
                                    xt[:, taps[0] : taps[0] + jw, :],
                                    fb(fc5[di], taps[0], jw),
                                    mult,
                                )
                                for t in taps[1:]:
                                    tb = td.tile(
                                        [128, JW, C], bf16, tag="tb", bufs=3
                                    )
                                    stt(
                                        eng,
                                        tb[:, :jw, :],
                                        xt[:, t : t + jw, :],
                                        fb(fc5[di], t, jw),
                                        mult,
                                    )
                                    g2 = td.tile(
                                        [128, JW, C], bf16, tag="g", bufs=6
                                    )
                                    stt(
                                        eng,
                                        g2[:, :jw, :],
                                        g[:, :jw, :],
                                        tb[:, :jw, :],
                                        add,
                                    )
                                    g = g2
                                for jj in range(0, jw, BANK_J):
                                    njw = min(BANK_J, jw - jj)
                                    nc.tensor.matmul(
                                        ps[:, jj : jj + njw, :],
                                        ident[:, di : di + 124],
                                        g[:, jj : jj + njw, :],
                                        start=(stream == 0),
                                        stop=(stream == N_STREAMS - 1),
                                    )
                                stream += 1
                        stage = sp.tile([124, JW, C], bf16, tag="stage")
                        nc.scalar.copy(out=stage[:, :jw, :], in_=ps[:, :jw, :])
                        nc.sync.dma_start(
                            out=out[i0 : i0 + 124, j0 : j0 + jw, :],
                            in_=stage[:, :jw, :],
                        )

                # --- strip: out rows [248,252), transposed (partition=j) ---
                for j0, P in ((0, 124), (124, 124), (248, 4)):
                    xs5 = []
                    for dj in range(K):
                        xs = fp.tile([P, 8, C], bf16, tag=f"sx{dj}")
                        nc.sync.dma_start(
                            out=xs,
                            in_=x[
                                HO - 4 : HO + 4, j0 + dj : j0 + dj + P, :
                            ].rearrange("r j c -> j r c"),
                        )
                        xs5.append(xs)
                    fs = fp.tile([P, 4, 2 * K * K], bf16, tag="sf")
                    nc.sync.dma_start(
                        out=fs,
                        in_=fd[HO - 4 : HO, j0 : j0 + P, :].rearrange(
                            "i j t -> j i t"
                        ),
                    )
                    ps_s = pp.tile([P, 4, C], f32, tag="ps")
                    for t in range(K * K):
                        di, dj = divmod(t, K)
                        eng = nc.gpsimd if dj >= 3 else nc.vector
                        tmp = td.tile([P, 4, C], bf16, tag="st", bufs=4)
                        fbs = (
                            fs[:, :, 2 * t : 2 * t + 2]
                            .unsqueeze(2)
                            .to_broadcast([P, 4, C // 2, 2])
                        )
                        stt(eng, tmp, xs5[dj][:, di : di + 4, :], fbs, mult)
                        nc.tensor.matmul(
                            ps_s[:, :, :],
                            ident[:P, :P],
                            tmp[:, :, :],
                            start=(t == 0),
                            stop=(t == K * K - 1),
                        )
                    stage = sp.tile([P, 4, C], bf16, tag="sstage")
                    nc.scalar.copy(out=stage, in_=ps_s)
                    nc.sync.dma_start(
                        out=out[HO - 4 : HO, j0 : j0 + P, :].rearrange(
                            "i j c -> j i c"
                        ),
                        in_=stage,
                    )

    nc.finalize()
    _nc_cache[reps] = nc
    return nc


def _to_bf16(a):
    import ml_dtypes

    return np.ascontiguousarray(a.astype(ml_dtypes.bfloat16))


def _core_inputs(x_core, flow_core):
    """f32 [H,W,C] and [HO,WO,25] -> bf16 input map for one core."""
    fd = np.repeat(_to_bf16(np.asarray(flow_core)), 2, axis=-1)
    return {"x": _to_bf16(np.asarray(x_core)), "fd": np.ascontiguousarray(fd)}


def _postprocess_core(out_core):
    return np.asarray(out_core, dtype=np.float32)


def _run(x, flow, trace=False):
    """x: [8,H,W,C] f32, flow: [8,HO,WO,25] f32 -> (out [8,HO,WO,C], res)"""
    from concourse.bass_utils import run_bass_kernel_spmd

    nc = _build()
    in_maps = [_core_inputs(x[b], flow[b]) for b in range(NCORES)]
    res = run_bass_kernel_spmd(
        nc, in_maps, core_ids=list(range(NCORES)), trace=trace
    )
    out = np.stack(
        [_postprocess_core(r["out"]) for r in res.results], axis=0
    )
    return out, res


def kernel(x, flow, ksize=None, **_unused):
    x = np.asarray(x, dtype=np.float32)
    flow = np.asarray(flow, dtype=np.float32)
    out, _ = _run(x, flow, trace=False)
    return out
